# revision 6
# baseline (speedup 1.0000x reference)
"""Trainium2 Bass kernel for ExcitationEmbedding + Ion RoPE.

Computes, for inputs
  excitations [256, 512, 2] int64 (pairs (a, b) with a, b in [0, 6)),
  n_electrons [256] f32, n_protons [256] f32,
  emb_weight  [26, 256] f32, lookup_table [6, 6] int64:

  idx   = lookup_table[a, b]                       # [B, N]
  emb   = emb_weight[idx]                          # [B, N, D]
  out   = per-batch block-diagonal rotation of emb (theta from n_electrons,
          phi from n_protons, 4-wide blocks: dims (0,1) by theta, (2,3) by phi)

Strategy (pure data parallel over 8 cores, 32 batches each):
  - Host packs each excitation pair into one int8 code a + 16*b, inverts
    lookup_table into a per-row code list, and pre-replicates the packed
    codes / angle inputs across the 4 partition blocks (pure input
    staging), so there are no on-device broadcasts or DRAM bounces at all.
  - 26-row work is packed x4 onto partition blocks {0,32,64,96} (block q
    holds batches 8q..8q+7).  The one-hot is a single is_equal (gpsimd);
    rotated tables rot[j,b,d] = e[j,d]*c(b,d) + e_sw[j,d]*s(b,d) come from
    sin() evaluated directly on all 128 partitions and two free-dim-
    broadcast vector muls.
  - Gather matmuls use PE row-group tiling: pair (b, b+16) lives in row
    groups q and q+2, and the 8 chunk matmuls of a pair are interleaved
    across the two groups so they execute concurrently in the PE array
    (tile_position row packing, ~2x at the pinned 1.2 GHz clock).
  - Each pair shares one 4-bank PSUM tile; the scalar engine evacuates
    batch b (banks 0-1) while the vector engine evacuates batch b+16
    (banks 2-3) concurrently, casting f32 -> fp16 (exact: the one-hot
    gather output is fp16 table rows).
  - fp16 output halves HBM write traffic; each batch's 256 KB linear DMA
    issues as soon as that batch is evacuated, so the HBM write stream
    starts as early as possible (it is the roofline: ~8.4 MB/core).
"""

import functools

import numpy as np

import concourse.bass as bass
import concourse.bacc as bacc
import concourse.mybir as mybir
from concourse import tile
from concourse.bass_utils import run_bass_kernel_spmd

B, N, D = 256, 512, 256
N_CORES = 8
BL = B // N_CORES   # 32 batches per core
ANGLE_SCALE = 0.05
HALF_PI = float(np.pi / 2)

F32 = mybir.dt.float32
F16 = mybir.dt.float16
I8 = mybir.dt.int8
AF = mybir.ActivationFunctionType
ALU = mybir.AluOpType


def build_bass() -> bass.Bass:
    nc = bacc.Bacc(
        "TRN2", target_bir_lowering=False, debug=False, num_devices=N_CORES
    )

    # host-replicated inputs: row 32q+j holds block q's batches 8q..8q+7
    exc = nc.dram_tensor("exc", [128, 8 * N], I8, kind="ExternalInput")
    ne = nc.dram_tensor("ne", [128, 8], F32, kind="ExternalInput")
    npr = nc.dram_tensor("npr", [128, 8], F32, kind="ExternalInput")
    emb = nc.dram_tensor("emb", [26, D], F32, kind="ExternalInput")
    codes = nc.dram_tensor("codes", [128, 1], F32, kind="ExternalInput")
    out = nc.dram_tensor("out", [BL, N, D], F16, kind="ExternalOutput")

    with tile.TileContext(nc) as tc:
        with (
            tc.tile_pool(name="const", bufs=1) as const,
            tc.tile_pool(name="opool", bufs=3) as opool,
            tc.tile_pool(name="psum", bufs=2, space="PSUM") as psum,
        ):
            # ---- input loads (sync queue; all early, all straight) ----
            ne_s = const.tile([128, 8], F32)
            nc.sync.dma_start(out=ne_s[:], in_=ne[:])
            npr_s = const.tile([128, 8], F32)
            nc.sync.dma_start(out=npr_s[:], in_=npr[:])
            exc_s = const.tile([128, 8, N], I8)
            nc.sync.dma_start(out=exc_s[:], in_=exc[:].rearrange(
                "p (r n) -> p r n", r=8))
            codes_s = const.tile([128, 1], F32)
            nc.sync.dma_start(out=codes_s[:], in_=codes[:])
            emb4_f = const.tile([128, D], F32)
            for q in range(4):
                nc.sync.dma_start(out=emb4_f[32 * q:32 * q + 26, :], in_=emb[:])

            # ---- angle patterns on all 128 partitions ----
            # ang[:, r, 0:4] = (ct,ct,cp,cp)(batch), ang[:, r, 4:8] = (st,-st,sp,-sp)
            hp = const.tile([128, 1], F32)
            nc.vector.memset(hp[:], HALF_PI)
            ang = const.tile([128, 8, 8], F16)
            # cos(t) = sin(pi/2 - t) keeps the LUT argument within [-pi, pi]
            for i, (src, bias, scale) in enumerate([
                    (ne_s, hp, -ANGLE_SCALE), (ne_s, hp, -ANGLE_SCALE),
                    (npr_s, hp, -ANGLE_SCALE), (npr_s, hp, -ANGLE_SCALE),
                    (ne_s, 0.0, ANGLE_SCALE), (ne_s, 0.0, -ANGLE_SCALE),
                    (npr_s, 0.0, ANGLE_SCALE), (npr_s, 0.0, -ANGLE_SCALE)]):
                b_ap = bias[:] if not isinstance(bias, float) else bias
                nc.scalar.activation(ang[:, :, i], src[:], AF.Sin,
                                     bias=b_ap, scale=scale)

            # ---- embedding rows on all 4 blocks: eA plain, eB pair-swapped
            eA = const.tile([128, D], F16)
            nc.vector.tensor_copy(eA[:], emb4_f[:])
            eB = const.tile([128, D], F16)
            eA2 = eA[:].rearrange("p (k i) -> p k i", i=2)
            eB2 = eB[:].rearrange("p (k i) -> p k i", i=2)
            nc.vector.tensor_copy(eB2[:, :, 0], eA2[:, :, 1])
            nc.vector.tensor_copy(eB2[:, :, 1], eA2[:, :, 0])

            # ---- one-hot + rotated tables, in two halves for pipelining ----
            oh = const.tile([128, 8, N], F16)
            t12a = const.tile([128, 8, 64, 4], F16)
            t12b = const.tile([128, 8, 64, 4], F16)
            rot = const.tile([128, 8, D], F16)
            rot4 = rot[:].rearrange("p r (k i) -> p r k i", i=4)
            eA4 = eA[:].rearrange("p (k i) -> p k i", i=4)
            eB4 = eB[:].rearrange("p (k i) -> p k i", i=4)
            for s in range(2):
                hs = slice(4 * s, 4 * s + 4)
                nc.gpsimd.tensor_scalar(out=oh[:, hs, :], in0=exc_s[:, hs, :],
                                        scalar1=codes_s[:], scalar2=None,
                                        op0=ALU.is_equal)
                nc.vector.tensor_mul(
                    t12a[:, hs, :, :],
                    eA4.unsqueeze(1).to_broadcast((128, 4, 64, 4)),
                    ang[:, hs, 0:4].unsqueeze(2).to_broadcast((128, 4, 64, 4)))
                nc.vector.tensor_mul(
                    t12b[:, hs, :, :],
                    eB4.unsqueeze(1).to_broadcast((128, 4, 64, 4)),
                    ang[:, hs, 4:8].unsqueeze(2).to_broadcast((128, 4, 64, 4)))
                nc.gpsimd.tensor_add(rot4[:, hs, :, :], t12a[:, hs, :, :],
                                     t12b[:, hs, :, :])

            # ---- gather matmuls (row-group paired) + split evacuation ----
            # pair (b, b+16): row groups q=b//8 and q+2 run concurrently
            for s in (0, 1):            # table half: batches-in-block 4s..4s+4
                for bb in range(8):
                    b = 8 * (bb // 4) + 4 * s + bb % 4   # pairs in half s
                    b2 = b + 16
                    q, rr = b // 8, b % 8
                    ps = psum.tile([128, 8 * D], F32, tag="ps", bufs=2)
                    for c in range(4):
                        # chunk c covers tokens {4k + c}
                        nc.tensor.matmul(
                            ps[:, c * D:(c + 1) * D],
                            oh[32 * q:32 * q + 26, rr, c::4],
                            rot[32 * q:32 * q + 26, rr, :],
                            start=True, stop=True, tile_position=(32 * q, 0))
                        nc.tensor.matmul(
                            ps[:, (4 + c) * D:(5 + c) * D],
                            oh[64 + 32 * q:64 + 32 * q + 26, rr, c::4],
                            rot[64 + 32 * q:64 + 32 * q + 26, rr, :],
                            start=True, stop=True,
                            tile_position=(64 + 32 * q, 0))
                    obuf = opool.tile([128, 2, 4 * D], F16, tag="obuf", bufs=3)
                    # scalar drains banks 0-1 (batch b) while vector drains
                    # banks 2-3 (batch b+16)
                    nc.scalar.activation(obuf[:, 0, :], ps[:, 0:4 * D], AF.Copy)
                    nc.vector.tensor_copy(obuf[:, 1, :], ps[:, 4 * D:8 * D])
                    # token t = 4k + c sits at obuf[k, j, c*256+d] -> linear
                    nc.sync.dma_start(
                        out=out[b].rearrange("(p c) d -> p (c d)", p=128),
                        in_=obuf[:, 0, :])
                    nc.sync.dma_start(
                        out=out[b2].rearrange("(p c) d -> p (c d)", p=128),
                        in_=obuf[:, 1, :])

    nc.compile()
    return nc


@functools.lru_cache(maxsize=1)
def _get_nc() -> bass.Bass:
    return build_bass()


def kernel_with_results(excitations, n_electrons, n_protons, emb_weight,
                        lookup_table, trace=False):
    exc = np.asarray(excitations)
    codes8 = (exc[..., 0] + 16 * exc[..., 1]).astype(np.int8)  # [B, N]
    ne = np.asarray(n_electrons, dtype=np.float32)
    npr = np.asarray(n_protons, dtype=np.float32)
    emb = np.ascontiguousarray(np.asarray(emb_weight, dtype=np.float32))
    lut = np.asarray(lookup_table)
    codes32 = np.full((32,), 1e9, dtype=np.float32)  # pad: never matches
    for x in range(6):
        for y in range(6):
            r = int(lut[x, y])
            if 0 <= r < 26:
                codes32[r] = float(x + 16 * y)
    codes128 = np.ascontiguousarray(np.tile(codes32, 4).reshape(128, 1))

    in_maps = []
    for c in range(N_CORES):
        sl = slice(c * BL, (c + 1) * BL)
        exc_c = codes8[sl].reshape(4, 8 * N)            # [4 blocks, 8*512]
        ne_c = ne[sl].reshape(4, 8)
        npr_c = npr[sl].reshape(4, 8)
        in_maps.append({
            "exc": np.ascontiguousarray(np.repeat(exc_c, 32, axis=0)),
            "ne": np.ascontiguousarray(np.repeat(ne_c, 32, axis=0)),
            "npr": np.ascontiguousarray(np.repeat(npr_c, 32, axis=0)),
            "emb": emb,
            "codes": codes128,
        })

    nc = _get_nc()
    res = run_bass_kernel_spmd(nc, in_maps, list(range(N_CORES)), trace=trace)
    out_arr = np.concatenate(
        [res.results[c]["out"] for c in range(N_CORES)], axis=0)
    return np.ascontiguousarray(out_arr.reshape(B, N, D).astype(np.float32)), res


def kernel(excitations, n_electrons, n_protons, emb_weight, lookup_table):
    out_arr, _ = kernel_with_results(excitations, n_electrons, n_protons,
                                     emb_weight, lookup_table)
    return out_arr


# revision 7
# speedup vs baseline: 2.3448x; 2.3448x over previous
"""Trainium2 Bass kernel for ExcitationEmbedding + Ion RoPE.

Computes, for inputs
  excitations [256, 512, 2] int64 (pairs (a, b) with a, b in [0, 6)),
  n_electrons [256] f32, n_protons [256] f32,
  emb_weight  [26, 256] f32, lookup_table [6, 6] int64:

  idx   = lookup_table[a, b]                       # [B, N]
  emb   = emb_weight[idx]                          # [B, N, D]
  out   = per-batch block-diagonal rotation of emb (theta from n_electrons,
          phi from n_protons, 4-wide blocks: dims (0,1) by theta, (2,3) by phi)

Strategy (pure data parallel over 8 cores, 32 batches each):
  - Host packs each excitation pair into one int8 code a + 16*b, inverts
    lookup_table into a per-row code list, and pre-replicates the packed
    codes / angle inputs across the 4 partition blocks (pure input
    staging), so there are no on-device broadcasts or DRAM bounces.
  - 26-row work is packed x4 onto partition blocks {0,32,64,96} (block q
    holds batches 8q..8q+7).  One vector is_equal per half builds the
    one-hots; sin/cos come from a fused-op polynomial on the vector
    engine (max err ~9e-5, under the fp16 noise floor), so the scalar
    engine's activation-table load never gates the critical path.
  - Gather matmuls use PE row-group tiling: pair (b, b+16) lives in row
    groups q and q+2 and its 8 chunk matmuls interleave across the two
    groups, executing concurrently in the PE array (~2x at the pinned
    1.2 GHz clock).
  - Each pair shares one 4-bank PSUM tile evacuated WHOLE by one engine
    (alternating scalar/vector per pair) - concurrent ACT+DVE reads of
    the same PSUM tile measurably contend, different tiles don't.
  - fp16 output (exact: the gather output is fp16 table rows) halves HBM
    writes; each batch's 256 KB linear DMA issues as soon as the batch
    is evacuated, so the 8.4 MB/core HBM write stream (the roofline)
    starts as early as possible.
"""

import functools

import numpy as np

import concourse.bass as bass
import concourse.bacc as bacc
import concourse.mybir as mybir
from concourse import tile
from concourse.bass_utils import run_bass_kernel_spmd

B, N, D = 256, 512, 256
N_CORES = 8
BL = B // N_CORES   # 32 batches per core
ANGLE_SCALE = 0.05

# sin(x) ~ x*(SA0 + SA1 t + SA2 t^2 + SA3 t^3), t = x^2, on [0, 2.5]
SA = (9.99904493e-01, -1.66440650e-01, 8.19030549e-03, -1.64793798e-04)
# cos(x) ~ CB0 + CB1 t + ... + CB4 t^4
CB = (9.99996835e-01, -4.99972001e-01, 4.16275457e-02, -1.36987157e-03,
      2.10124017e-05)

F32 = mybir.dt.float32
F16 = mybir.dt.float16
I8 = mybir.dt.int8
AF = mybir.ActivationFunctionType
ALU = mybir.AluOpType


def build_bass() -> bass.Bass:
    nc = bacc.Bacc(
        "TRN2", target_bir_lowering=False, debug=False, num_devices=N_CORES
    )

    # host-replicated inputs: row 32q+j holds block q's batches 8q..8q+7
    exc = nc.dram_tensor("exc", [128, 8 * N], I8, kind="ExternalInput")
    ne = nc.dram_tensor("ne", [128, 8], F32, kind="ExternalInput")
    npr = nc.dram_tensor("npr", [128, 8], F32, kind="ExternalInput")
    emb = nc.dram_tensor("emb", [26, D], F32, kind="ExternalInput")
    codes = nc.dram_tensor("codes", [128, 1], F32, kind="ExternalInput")
    out = nc.dram_tensor("out", [BL, N, D], F16, kind="ExternalOutput")

    with tile.TileContext(nc) as tc:
        with (
            tc.tile_pool(name="const", bufs=1) as const,
            tc.tile_pool(name="opool", bufs=3) as opool,
            tc.tile_pool(name="psum", bufs=2, space="PSUM") as psum,
        ):
            # ---- input loads (sync queue; all early, all straight) ----
            ne_s = const.tile([128, 8], F32)
            nc.sync.dma_start(out=ne_s[:], in_=ne[:])
            npr_s = const.tile([128, 8], F32)
            nc.sync.dma_start(out=npr_s[:], in_=npr[:])
            exc_s = const.tile([128, 8, N], I8)
            nc.sync.dma_start(out=exc_s[:], in_=exc[:].rearrange(
                "p (r n) -> p r n", r=8))
            codes_s = const.tile([128, 1], F32)
            nc.sync.dma_start(out=codes_s[:], in_=codes[:])
            emb4_f = const.tile([128, D], F32)
            for q in range(4):
                nc.sync.dma_start(out=emb4_f[32 * q:32 * q + 26, :], in_=emb[:])

            # ---- prefetch the scalar engine's activation table (Copy set)
            # so the first PSUM evacuation doesn't pay the table load
            scratch = const.tile([128, 1], F32)
            nc.vector.memset(scratch[:], 0.0)
            nc.scalar.activation(scratch[:], scratch[:], AF.Copy)

            # ---- one-hot, half 0 first (is_equal is the earliest-ready op)
            oh = const.tile([128, 8, N], F16)
            nc.vector.tensor_scalar(out=oh[:, 0:4, :], in0=exc_s[:, 0:4, :],
                                    scalar1=codes_s[:], scalar2=None,
                                    op0=ALU.is_equal)

            # ---- angle patterns via polynomial sin/cos (vector engine) ----
            # ang[:, r, 0:4] = (ct,ct,cp,cp), ang[:, r, 4:8] = (st,-st,sp,-sp)
            ang = const.tile([128, 8, 8], F16)
            for src, cos_i, sin_i in ((ne_s, 0, 4), (npr_s, 2, 6)):
                th = const.tile([128, 8], F32)
                nc.vector.tensor_scalar(out=th[:], in0=src[:],
                                        scalar1=ANGLE_SCALE, scalar2=None,
                                        op0=ALU.mult)
                t2 = const.tile([128, 8], F32)
                nc.vector.tensor_mul(t2[:], th[:], th[:])
                v = const.tile([128, 8], F32)
                nc.vector.tensor_scalar(out=v[:], in0=t2[:], scalar1=SA[3],
                                        scalar2=None, op0=ALU.mult)
                nc.vector.scalar_tensor_tensor(out=v[:], in0=v[:],
                                               scalar=SA[2], in1=t2[:],
                                               op0=ALU.add, op1=ALU.mult)
                nc.vector.scalar_tensor_tensor(out=v[:], in0=v[:],
                                               scalar=SA[1], in1=t2[:],
                                               op0=ALU.add, op1=ALU.mult)
                nc.vector.scalar_tensor_tensor(out=ang[:, :, sin_i], in0=v[:],
                                               scalar=SA[0], in1=th[:],
                                               op0=ALU.add, op1=ALU.mult)
                nc.vector.tensor_scalar(out=ang[:, :, sin_i + 1],
                                        in0=ang[:, :, sin_i], scalar1=-1.0,
                                        scalar2=None, op0=ALU.mult)
                w = const.tile([128, 8], F32)
                nc.vector.tensor_scalar(out=w[:], in0=t2[:], scalar1=CB[4],
                                        scalar2=None, op0=ALU.mult)
                nc.vector.scalar_tensor_tensor(out=w[:], in0=w[:],
                                               scalar=CB[3], in1=t2[:],
                                               op0=ALU.add, op1=ALU.mult)
                nc.vector.scalar_tensor_tensor(out=w[:], in0=w[:],
                                               scalar=CB[2], in1=t2[:],
                                               op0=ALU.add, op1=ALU.mult)
                nc.vector.scalar_tensor_tensor(out=w[:], in0=w[:],
                                               scalar=CB[1], in1=t2[:],
                                               op0=ALU.add, op1=ALU.mult)
                nc.vector.tensor_scalar(out=ang[:, :, cos_i], in0=w[:],
                                        scalar1=CB[0], scalar2=None,
                                        op0=ALU.add)
                nc.vector.tensor_copy(ang[:, :, cos_i + 1], ang[:, :, cos_i])

            # ---- embedding rows on all 4 blocks: eA plain, eB pair-swapped
            eA = const.tile([128, D], F16)
            nc.vector.tensor_copy(eA[:], emb4_f[:])
            eB = const.tile([128, D], F16)
            eA2 = eA[:].rearrange("p (k i) -> p k i", i=2)
            eB2 = eB[:].rearrange("p (k i) -> p k i", i=2)
            nc.vector.tensor_copy(eB2[:, :, 0], eA2[:, :, 1])
            nc.vector.tensor_copy(eB2[:, :, 1], eA2[:, :, 0])

            # ---- rotated tables (half 0 now, half 1 emitted mid-loop) ----
            t12a = const.tile([128, 8, 64, 4], F16)
            t12b = const.tile([128, 8, 64, 4], F16)
            rot = const.tile([128, 8, D], F16)
            rot4 = rot[:].rearrange("p r (k i) -> p r k i", i=4)
            eA4 = eA[:].rearrange("p (k i) -> p k i", i=4)
            eB4 = eB[:].rearrange("p (k i) -> p k i", i=4)

            def table_half(s):
                hs = slice(4 * s, 4 * s + 4)
                if s == 1:
                    nc.vector.tensor_scalar(out=oh[:, hs, :],
                                            in0=exc_s[:, hs, :],
                                            scalar1=codes_s[:], scalar2=None,
                                            op0=ALU.is_equal)
                nc.vector.tensor_mul(
                    t12a[:, hs, :, :],
                    eA4.unsqueeze(1).to_broadcast((128, 4, 64, 4)),
                    ang[:, hs, 0:4].unsqueeze(2).to_broadcast((128, 4, 64, 4)))
                nc.vector.tensor_mul(
                    t12b[:, hs, :, :],
                    eB4.unsqueeze(1).to_broadcast((128, 4, 64, 4)),
                    ang[:, hs, 4:8].unsqueeze(2).to_broadcast((128, 4, 64, 4)))
                nc.vector.tensor_add(rot4[:, hs, :, :], t12a[:, hs, :, :],
                                     t12b[:, hs, :, :])

            table_half(0)

            # ---- gather matmuls (row-group paired) + alternating evac ----
            # pair (b, b+16): row groups q and q+2 run concurrently
            pair_idx = 0
            for s in (0, 1):            # table half: batches-in-block 4s..4s+4
                for bb in range(8):
                    b = 8 * (bb // 4) + 4 * s + bb % 4
                    b2 = b + 16
                    q, rr = b // 8, b % 8
                    ps = psum.tile([128, 8 * D], F32, tag="ps", bufs=2)
                    for c in range(4):
                        # chunk c covers tokens {4k + c}
                        nc.tensor.matmul(
                            ps[:, c * D:(c + 1) * D],
                            oh[32 * q:32 * q + 26, rr, c::4],
                            rot[32 * q:32 * q + 26, rr, :],
                            start=True, stop=True, tile_position=(32 * q, 0))
                        nc.tensor.matmul(
                            ps[:, (4 + c) * D:(5 + c) * D],
                            oh[64 + 32 * q:64 + 32 * q + 26, rr, c::4],
                            rot[64 + 32 * q:64 + 32 * q + 26, rr, :],
                            start=True, stop=True,
                            tile_position=(64 + 32 * q, 0))
                    obuf = opool.tile([128, 2, 4 * D], F16, tag="obuf", bufs=3)
                    ps2 = ps[:].rearrange("p (j f) -> p j f", j=2)
                    if pair_idx % 2 == 0:
                        nc.scalar.activation(obuf[:], ps2, AF.Copy)
                    else:
                        nc.vector.tensor_copy(obuf[:], ps2)
                    # token t = 4k + c sits at obuf[k, j, c*256+d] -> linear
                    nc.sync.dma_start(
                        out=out[b].rearrange("(p c) d -> p (c d)", p=128),
                        in_=obuf[:, 0, :])
                    nc.sync.dma_start(
                        out=out[b2].rearrange("(p c) d -> p (c d)", p=128),
                        in_=obuf[:, 1, :])
                    pair_idx += 1
                    if s == 0 and bb == 1:
                        table_half(1)   # overlap half-1 tables with the loop

    nc.compile()
    return nc


@functools.lru_cache(maxsize=1)
def _get_nc() -> bass.Bass:
    return build_bass()


def kernel_with_results(excitations, n_electrons, n_protons, emb_weight,
                        lookup_table, trace=False):
    exc = np.asarray(excitations)
    codes8 = (exc[..., 0] + 16 * exc[..., 1]).astype(np.int8)  # [B, N]
    ne = np.asarray(n_electrons, dtype=np.float32)
    npr = np.asarray(n_protons, dtype=np.float32)
    emb = np.ascontiguousarray(np.asarray(emb_weight, dtype=np.float32))
    lut = np.asarray(lookup_table)
    codes32 = np.full((32,), 1e9, dtype=np.float32)  # pad: never matches
    for x in range(6):
        for y in range(6):
            r = int(lut[x, y])
            if 0 <= r < 26:
                codes32[r] = float(x + 16 * y)
    codes128 = np.ascontiguousarray(np.tile(codes32, 4).reshape(128, 1))

    in_maps = []
    for c in range(N_CORES):
        sl = slice(c * BL, (c + 1) * BL)
        exc_c = codes8[sl].reshape(4, 8 * N)            # [4 blocks, 8*512]
        ne_c = ne[sl].reshape(4, 8)
        npr_c = npr[sl].reshape(4, 8)
        in_maps.append({
            "exc": np.ascontiguousarray(np.repeat(exc_c, 32, axis=0)),
            "ne": np.ascontiguousarray(np.repeat(ne_c, 32, axis=0)),
            "npr": np.ascontiguousarray(np.repeat(npr_c, 32, axis=0)),
            "emb": emb,
            "codes": codes128,
        })

    nc = _get_nc()
    res = run_bass_kernel_spmd(nc, in_maps, list(range(N_CORES)), trace=trace)
    out_arr = np.concatenate(
        [res.results[c]["out"] for c in range(N_CORES)], axis=0)
    return np.ascontiguousarray(out_arr.reshape(B, N, D).astype(np.float32)), res


def kernel(excitations, n_electrons, n_protons, emb_weight, lookup_table):
    out_arr, _ = kernel_with_results(excitations, n_electrons, n_protons,
                                     emb_weight, lookup_table)
    return out_arr


# revision 8
# speedup vs baseline: 2.7588x; 1.1766x over previous
"""Trainium2 Bass kernel for ExcitationEmbedding + Ion RoPE.

Computes, for inputs
  excitations [256, 512, 2] int64 (pairs (a, b) with a, b in [0, 6)),
  n_electrons [256] f32, n_protons [256] f32,
  emb_weight  [26, 256] f32, lookup_table [6, 6] int64:

  idx   = lookup_table[a, b]                       # [B, N]
  emb   = emb_weight[idx]                          # [B, N, D]
  out   = per-batch block-diagonal rotation of emb (theta from n_electrons,
          phi from n_protons, 4-wide blocks: dims (0,1) by theta, (2,3) by phi)

Strategy (pure data parallel over 8 cores, 32 batches each):
  - Host packs each excitation pair into one int8 code a + 16*b, inverts
    lookup_table into a per-row code list, and pre-replicates the packed
    codes / angle inputs / embedding across the 4 partition blocks (pure
    input staging), so every load is a single straight DMA and there are
    no on-device broadcasts or DRAM bounces.
  - 26-row work is packed x4 onto partition blocks {0,32,64,96} (block q
    holds batches 8q..8q+7).  One vector is_equal per half builds the
    one-hots; sin/cos come from a fused-op polynomial on the vector
    engine (max err ~9e-5, under the fp16 noise floor), keeping the
    scalar engine's activation-table load off the critical path.
  - Gather matmuls use PE row-group tiling: pair (b, b+16) lives in row
    groups q and q+2 and its 8 chunk matmuls interleave across the two
    groups, executing concurrently in the PE array (~2x at the pinned
    1.2 GHz clock).
  - Each batch gets its own 2-bank PSUM tile (4 buffers = all 8 banks):
    the scalar engine evacuates batch b while the vector engine
    evacuates batch b+16 concurrently on different tiles (same-tile
    concurrent reads contend; different tiles run at full speed).
  - fp16 output (exact: the gather output is fp16 table rows) halves HBM
    writes; each batch's 256 KB linear DMA issues as soon as the batch
    is evacuated, so the 8.4 MB/core HBM write stream (the roofline)
    starts as early as possible.
"""

import functools

import numpy as np

import concourse.bass as bass
import concourse.bacc as bacc
import concourse.mybir as mybir
from concourse import tile
from concourse.bass_utils import run_bass_kernel_spmd

B, N, D = 256, 512, 256
N_CORES = 8
BL = B // N_CORES   # 32 batches per core
ANGLE_SCALE = 0.05

# sin(x) ~ x*(SA0 + SA1 t + SA2 t^2 + SA3 t^3), t = x^2, on [0, 2.5]
SA = (9.99904493e-01, -1.66440650e-01, 8.19030549e-03, -1.64793798e-04)
# cos(x) ~ CB0 + CB1 t + ... + CB4 t^4
CB = (9.99996835e-01, -4.99972001e-01, 4.16275457e-02, -1.36987157e-03,
      2.10124017e-05)

F32 = mybir.dt.float32
F16 = mybir.dt.float16
I8 = mybir.dt.int8
AF = mybir.ActivationFunctionType
ALU = mybir.AluOpType


def build_bass() -> bass.Bass:
    nc = bacc.Bacc(
        "TRN2", target_bir_lowering=False, debug=False, num_devices=N_CORES
    )

    # host-replicated inputs: row 32q+j holds block q's batches 8q..8q+7
    nepr = nc.dram_tensor("nepr", [128, 16], F32, kind="ExternalInput")
    emb = nc.dram_tensor("emb", [128, D], F32, kind="ExternalInput")
    exc = nc.dram_tensor("exc", [128, 8 * N], I8, kind="ExternalInput")
    codes = nc.dram_tensor("codes", [128, 1], F32, kind="ExternalInput")
    out = nc.dram_tensor("out", [BL, N, D], F16, kind="ExternalOutput")

    with tile.TileContext(nc) as tc:
        with (
            tc.tile_pool(name="const", bufs=1) as const,
            tc.tile_pool(name="opool", bufs=6) as opool,
            tc.tile_pool(name="psum", bufs=4, space="PSUM") as psum,
        ):
            # ---- input loads (sync queue, critical-path order) ----
            nepr_s = const.tile([128, 16], F32)
            nc.sync.dma_start(out=nepr_s[:], in_=nepr[:])
            emb_s = const.tile([128, D], F32)
            nc.sync.dma_start(out=emb_s[:], in_=emb[:])
            exc_s = const.tile([128, 8, N], I8)
            nc.sync.dma_start(out=exc_s[:], in_=exc[:].rearrange(
                "p (r n) -> p r n", r=8))
            codes_s = const.tile([128, 1], F32)
            nc.sync.dma_start(out=codes_s[:], in_=codes[:])

            # ---- prefetch the scalar engine's activation table (Copy set)
            scratch = const.tile([128, 1], F32)
            nc.vector.memset(scratch[:], 0.0)
            nc.scalar.activation(scratch[:], scratch[:], AF.Copy)

            # ---- angle patterns via polynomial sin/cos, theta & phi fused
            # ang[:, r, 0:4] = (ct,ct,cp,cp), ang[:, r, 4:8] = (st,-st,sp,-sp)
            ang = const.tile([128, 8, 8], F16)
            th = const.tile([128, 16], F32)
            nc.vector.tensor_scalar(out=th[:], in0=nepr_s[:],
                                    scalar1=ANGLE_SCALE, scalar2=None,
                                    op0=ALU.mult)
            t2 = const.tile([128, 16], F32)
            nc.vector.tensor_mul(t2[:], th[:], th[:])
            v = const.tile([128, 16], F32)
            nc.vector.tensor_scalar(out=v[:], in0=t2[:], scalar1=SA[3],
                                    scalar2=None, op0=ALU.mult)
            nc.vector.scalar_tensor_tensor(out=v[:], in0=v[:], scalar=SA[2],
                                           in1=t2[:], op0=ALU.add,
                                           op1=ALU.mult)
            nc.vector.scalar_tensor_tensor(out=v[:], in0=v[:], scalar=SA[1],
                                           in1=t2[:], op0=ALU.add,
                                           op1=ALU.mult)
            # sin finals: theta -> slot 4, phi -> slot 6; then negations
            nc.vector.scalar_tensor_tensor(out=ang[:, :, 4], in0=v[:, 0:8],
                                           scalar=SA[0], in1=th[:, 0:8],
                                           op0=ALU.add, op1=ALU.mult)
            nc.vector.scalar_tensor_tensor(out=ang[:, :, 6], in0=v[:, 8:16],
                                           scalar=SA[0], in1=th[:, 8:16],
                                           op0=ALU.add, op1=ALU.mult)
            nc.vector.tensor_scalar(out=ang[:, :, 5], in0=ang[:, :, 4],
                                    scalar1=-1.0, scalar2=None, op0=ALU.mult)
            nc.vector.tensor_scalar(out=ang[:, :, 7], in0=ang[:, :, 6],
                                    scalar1=-1.0, scalar2=None, op0=ALU.mult)
            w = const.tile([128, 16], F32)
            nc.vector.tensor_scalar(out=w[:], in0=t2[:], scalar1=CB[4],
                                    scalar2=None, op0=ALU.mult)
            nc.vector.scalar_tensor_tensor(out=w[:], in0=w[:], scalar=CB[3],
                                           in1=t2[:], op0=ALU.add,
                                           op1=ALU.mult)
            nc.vector.scalar_tensor_tensor(out=w[:], in0=w[:], scalar=CB[2],
                                           in1=t2[:], op0=ALU.add,
                                           op1=ALU.mult)
            nc.vector.scalar_tensor_tensor(out=w[:], in0=w[:], scalar=CB[1],
                                           in1=t2[:], op0=ALU.add,
                                           op1=ALU.mult)
            # cos finals: theta -> slot 0, phi -> slot 2; then duplicates
            nc.vector.tensor_scalar(out=ang[:, :, 0], in0=w[:, 0:8],
                                    scalar1=CB[0], scalar2=None, op0=ALU.add)
            nc.vector.tensor_scalar(out=ang[:, :, 2], in0=w[:, 8:16],
                                    scalar1=CB[0], scalar2=None, op0=ALU.add)
            nc.vector.tensor_copy(ang[:, :, 1], ang[:, :, 0])
            nc.vector.tensor_copy(ang[:, :, 3], ang[:, :, 2])

            # ---- embedding rows: eA plain, eB pair-swapped ----
            eA = const.tile([128, D], F16)
            nc.vector.tensor_copy(eA[:], emb_s[:])
            eB = const.tile([128, D], F16)
            eA2 = eA[:].rearrange("p (k i) -> p k i", i=2)
            eB2 = eB[:].rearrange("p (k i) -> p k i", i=2)
            nc.vector.tensor_copy(eB2[:, :, 0], eA2[:, :, 1])
            nc.vector.tensor_copy(eB2[:, :, 1], eA2[:, :, 0])

            # ---- one-hot + rotated tables ----
            oh = const.tile([128, 8, N], F16)
            t12a = const.tile([128, 8, 64, 4], F16)
            t12b = const.tile([128, 8, 64, 4], F16)
            rot = const.tile([128, 8, D], F16)
            rot4 = rot[:].rearrange("p r (k i) -> p r k i", i=4)
            eA4 = eA[:].rearrange("p (k i) -> p k i", i=4)
            eB4 = eB[:].rearrange("p (k i) -> p k i", i=4)

            def table_part(lo, hi):
                hs = slice(lo, hi)
                n = hi - lo
                nc.vector.tensor_mul(
                    t12a[:, hs, :, :],
                    eA4.unsqueeze(1).to_broadcast((128, n, 64, 4)),
                    ang[:, hs, 0:4].unsqueeze(2).to_broadcast((128, n, 64, 4)))
                nc.vector.tensor_mul(
                    t12b[:, hs, :, :],
                    eB4.unsqueeze(1).to_broadcast((128, n, 64, 4)),
                    ang[:, hs, 4:8].unsqueeze(2).to_broadcast((128, n, 64, 4)))
                nc.vector.tensor_add(rot4[:, hs, :, :], t12a[:, hs, :, :],
                                     t12b[:, hs, :, :])
                nc.vector.tensor_scalar(out=oh[:, hs, :], in0=exc_s[:, hs, :],
                                        scalar1=codes_s[:], scalar2=None,
                                        op0=ALU.is_equal)

            table_part(0, 4)            # half 0: tables first, then one-hot

            # ---- gather matmuls (row-group paired) + concurrent evac ----
            # pair (b, b+16): row groups q and q+2 run concurrently on PE;
            # scalar evacuates b's tile while vector evacuates b+16's.
            for s in (0, 1):            # table half: batches-in-block 4s..4s+4
                for bb in range(8):
                    b = 8 * (bb // 4) + 4 * s + bb % 4
                    b2 = b + 16
                    q, rr = b // 8, b % 8
                    psA = psum.tile([128, 4 * D], F32, tag="ps", bufs=4)
                    psB = psum.tile([128, 4 * D], F32, tag="ps", bufs=4)
                    for c in range(4):
                        # chunk c covers tokens {4k + c}
                        nc.tensor.matmul(
                            psA[:, c * D:(c + 1) * D],
                            oh[32 * q:32 * q + 26, rr, c::4],
                            rot[32 * q:32 * q + 26, rr, :],
                            start=True, stop=True, tile_position=(32 * q, 0))
                        nc.tensor.matmul(
                            psB[:, c * D:(c + 1) * D],
                            oh[64 + 32 * q:64 + 32 * q + 26, rr, c::4],
                            rot[64 + 32 * q:64 + 32 * q + 26, rr, :],
                            start=True, stop=True,
                            tile_position=(64 + 32 * q, 0))
                    obA = opool.tile([128, 4 * D], F16, tag="obuf", bufs=6)
                    obB = opool.tile([128, 4 * D], F16, tag="obuf", bufs=6)
                    nc.scalar.activation(obA[:], psA[:], AF.Copy)
                    nc.vector.tensor_copy(obB[:], psB[:])
                    # token t = 4k + c sits at ob[k, c*256+d] -> linear 256 KB
                    nc.sync.dma_start(
                        out=out[b].rearrange("(p c) d -> p (c d)", p=128),
                        in_=obA[:])
                    nc.sync.dma_start(
                        out=out[b2].rearrange("(p c) d -> p (c d)", p=128),
                        in_=obB[:])
                    if s == 0 and bb == 1:
                        table_part(4, 6)    # overlap half-1 tables (part 1)
                    if s == 0 and bb == 3:
                        table_part(6, 8)    # overlap half-1 tables (part 2)

    nc.compile()
    return nc


@functools.lru_cache(maxsize=1)
def _get_nc() -> bass.Bass:
    return build_bass()


def kernel_with_results(excitations, n_electrons, n_protons, emb_weight,
                        lookup_table, trace=False):
    exc = np.asarray(excitations)
    codes8 = (exc[..., 0] + 16 * exc[..., 1]).astype(np.int8)  # [B, N]
    ne = np.asarray(n_electrons, dtype=np.float32)
    npr = np.asarray(n_protons, dtype=np.float32)
    emb = np.ascontiguousarray(np.asarray(emb_weight, dtype=np.float32))
    lut = np.asarray(lookup_table)
    codes32 = np.full((32,), 1e9, dtype=np.float32)  # pad: never matches
    for x in range(6):
        for y in range(6):
            r = int(lut[x, y])
            if 0 <= r < 26:
                codes32[r] = float(x + 16 * y)
    codes128 = np.ascontiguousarray(np.tile(codes32, 4).reshape(128, 1))
    emb128 = np.zeros((128, D), dtype=np.float32)
    for q in range(4):
        emb128[32 * q:32 * q + 26] = emb

    in_maps = []
    for c in range(N_CORES):
        sl = slice(c * BL, (c + 1) * BL)
        exc_c = codes8[sl].reshape(4, 8 * N)            # [4 blocks, 8*512]
        nepr_c = np.concatenate(
            [ne[sl].reshape(4, 8), npr[sl].reshape(4, 8)], axis=1)
        in_maps.append({
            "exc": np.ascontiguousarray(np.repeat(exc_c, 32, axis=0)),
            "nepr": np.ascontiguousarray(np.repeat(nepr_c, 32, axis=0)),
            "emb": emb128,
            "codes": codes128,
        })

    nc = _get_nc()
    res = run_bass_kernel_spmd(nc, in_maps, list(range(N_CORES)), trace=trace)
    out_arr = np.concatenate(
        [res.results[c]["out"] for c in range(N_CORES)], axis=0)
    return np.ascontiguousarray(out_arr.reshape(B, N, D).astype(np.float32)), res


def kernel(excitations, n_electrons, n_protons, emb_weight, lookup_table):
    out_arr, _ = kernel_with_results(excitations, n_electrons, n_protons,
                                     emb_weight, lookup_table)
    return out_arr
